# revision 1
# baseline (speedup 1.0000x reference)
"""BEV->Cylinder bilinear ring-sampling kernel for 8 Trainium2 NeuronCores.

Strategy (per core, 64 of the 512 (b,c) planes):
  * The 2048 sample points lie on a circle of radius 255.5 px; only a thin
    ring of the 512x512 BEV image is ever read. A host-precomputed rectangle
    cover of that ring (~19K px) is DMA'd into SBUF as X[plane, ring_px].
  * Each 128-px ring chunk is PE-transposed to XT[ring_px, plane].
  * col[plane, p] = sum_k w_k * I[corner_k(p)] is computed as a sequence of
    small PE matmuls col[:, window] += XT_chunk.T @ S_chunk where S is a
    host-precomputed sparse-in-dense weight matrix (bilinear weights at the
    ring positions of each p's corners), accumulated in PSUM.
  * col is copied to SBUF and broadcast over the 64 z-bins directly by the
    output DMAs (one per z-slab), writing the full [64, 64, 2048] shard.
All geometry/weights are input-independent compile-time constants baked into
the NEFF. Input dtype f32 is preserved end to end.
"""
import json
import math

import numpy as np

B, C, H_B, W_B = 4, 128, 512, 512
H_C, W_C = 64, 2048
MAX_RANGE = 50.0
XMIN, XMAX, YMIN, YMAX = -50.0, 50.0, -50.0, 50.0
NCORES = 8
PLANES = B * C // NCORES  # 64 planes per core

BAND = 16         # rows per cover band
CLUSTER_GAP = 16  # split x-clusters when gap exceeds this
MIN_W = 16        # min rect width (64B DMA bursts)
CHUNK = 128       # ring pixels per PE-transpose chunk
P_GAP = 48        # split p-interval when gap exceeds this
NQ = 4            # output column quarters

_CACHE = {}


# ----------------------------------------------------------------- geometry
def _sample_xy():
    """Sampling pixel coords exactly like the reference (jnp on CPU);
    numpy fallback differs only at ULP level."""
    try:
        import jax
        import jax.numpy as jnp
        cpu = jax.devices("cpu")[0]
        with jax.default_device(cpu):
            phi = jnp.linspace(-math.pi, math.pi, W_C)
            x_g = MAX_RANGE * jnp.cos(phi)
            y_g = MAX_RANGE * jnp.sin(phi)
            x = (x_g - XMIN) / (XMAX - XMIN) * (W_B - 1)
            y = (YMAX - y_g) / (YMAX - YMIN) * (H_B - 1)
            return np.asarray(x, np.float32), np.asarray(y, np.float32)
    except Exception:
        phi = np.linspace(-math.pi, math.pi, W_C, dtype=np.float32)
        x_g = (MAX_RANGE * np.cos(phi)).astype(np.float32)
        y_g = (MAX_RANGE * np.sin(phi)).astype(np.float32)
        x = ((x_g - XMIN) / (XMAX - XMIN) * (W_B - 1)).astype(np.float32)
        y = ((YMAX - y_g) / (YMAX - YMIN) * (H_B - 1)).astype(np.float32)
        return x, y


def _corners():
    x, y = _sample_xy()
    x0 = np.floor(x).astype(np.int64)
    y0 = np.floor(y).astype(np.int64)
    wx1 = (x - x0.astype(np.float32)).astype(np.float32)
    wx0 = (np.float32(1.0) - wx1).astype(np.float32)
    wy1 = (y - y0.astype(np.float32)).astype(np.float32)
    wy0 = (np.float32(1.0) - wy1).astype(np.float32)
    out = []
    for xi, wx in ((x0, wx0), (x0 + 1, wx1)):
        for yi, wy in ((y0, wy0), (y0 + 1, wy1)):
            w = (wx * wy).astype(np.float32)
            valid = (xi >= 0) & (xi < W_B) & (yi >= 0) & (yi < H_B)
            for p in range(W_C):
                if valid[p]:
                    out.append((p, int(yi[p]), int(xi[p]), float(w[p])))
    return out


def build_plan():
    corner_list = _corners()

    # rectangle cover of the ring, per 8-row band
    need = {}
    for p, yy, xx, w in corner_list:
        need.setdefault(yy // BAND, set()).add(xx)
    row_of_band = {}
    for p, yy, xx, w in corner_list:
        row_of_band.setdefault(yy // BAND, set()).add(yy)
    rects = []
    for b in sorted(need):
        xs = sorted(need[b])
        rows = sorted(row_of_band[b])
        ya, yb = min(rows), max(rows) + 1
        cl_start = prev = xs[0]
        for xx in xs[1:] + [None]:
            if xx is None or xx - prev > CLUSTER_GAP:
                xa, xbb = cl_start, prev + 1
                if xbb - xa < MIN_W:
                    xbb = min(xa + MIN_W, W_B)
                    xa = xbb - MIN_W
                rects.append((ya, yb - ya, xa, xbb - xa))
                if xx is not None:
                    cl_start = xx
            if xx is not None:
                prev = xx

    # order rects along the sampling circle (p order) so downstream
    # chunks/windows complete in p order and output DMAs unblock early
    import math as _math
    def _rect_p(rc):
        ya, h, xa, w = rc
        yc, xc = ya + h / 2.0, xa + w / 2.0
        phi = _math.atan2(255.5 - yc, xc - 255.5)
        return (phi + _math.pi) / (2 * _math.pi)
    rects.sort(key=_rect_p)

    # split rect list into two halves (partitions 0-63 / 64-127), each
    # padded to a CHUNK multiple
    areas = [h * w for (ya, h, xa, w) in rects]
    total = sum(areas)
    cum, split = 0, len(rects)
    for i, a in enumerate(areas):
        cum += a
        if cum >= total / 2:
            split = i + 1
            break
    halves = [rects[:split], rects[split:]]
    rect_dmas = []   # (half, local_off, ya, h, xa, w)
    half_len = []    # padded length of each half
    pix2ring = {}    # (y, x) -> global ring position
    half_base = [0, 0]
    glob_rects = []
    off_g = 0
    for hh, rl in enumerate(halves):
        off = 0
        half_base[hh] = off_g
        for (ya, h, xa, w) in rl:
            rect_dmas.append((hh, off, ya, h, xa, w))
            for r in range(h):
                for ccc in range(w):
                    key = (ya + r, xa + ccc)
                    if key not in pix2ring:
                        pix2ring[key] = half_base[hh] + off + r * w + ccc
            off += h * w
            off_g += h * w
        pad = (-off) % CHUNK
        memset_from = off
        off += pad
        off_g += pad
        half_len.append(off)
    hl = max(half_len)
    nchunks = (half_len[0] + half_len[1]) // CHUNK
    assert half_len[0] % CHUNK == 0 and half_len[1] % CHUNK == 0

    def chunk_loc(c):
        g = c * CHUNK
        if g < half_len[0]:
            return 0, g
        return 1, g - half_len[0]

    # S weights per chunk / p-interval
    chunk_hits = [dict() for _ in range(nchunks)]
    for p, yy, xx, w in corner_list:
        rp = pix2ring[(yy, xx)]
        d = chunk_hits[rp // CHUNK].setdefault(p, {})
        r = rp % CHUNK
        d[r] = d.get(r, 0.0) + w
    plan_mms = []
    s_cols = []
    s_off = 0
    for c in range(nchunks):
        if not chunk_hits[c]:
            continue
        ps = sorted(chunk_hits[c])
        st = prev = ps[0]
        ivs = []
        for p in ps[1:] + [None]:
            if p is None or p - prev > P_GAP:
                ivs.append((st, prev - st + 1))
                if p is not None:
                    st = p
            if p is not None:
                prev = p
        QB = W_C // NQ
        split_ivs = []
        for (pst, plen) in ivs:
            a = pst
            while a < pst + plen:
                b = min(pst + plen, (a // QB + 1) * QB)
                split_ivs.append((a, b - a))
                a = b
        for (pst, plen) in split_ivs:
            plan_mms.append((c, pst, plen, s_off))
            for p in range(pst, pst + plen):
                s_cols.append((c, p, chunk_hits[c].get(p, {})))
            s_off += plen
    S = np.zeros((CHUNK, s_off), dtype=np.float32)
    for j, (c, p, rows) in enumerate(s_cols):
        for r, w in rows.items():
            S[r, j] += np.float32(w)

    used = sorted({c for (c, _, _, _) in plan_mms})
    slot = {c: i for i, c in enumerate(used)}

    return dict(rect_dmas=rect_dmas, half_len=half_len, hl=hl,
                nchunks=nchunks, chunk_loc=chunk_loc, S=S,
                plan_mms=plan_mms, sum_m=s_off, used=used, slot=slot)


# ------------------------------------------------------- walrus wait-split
def split_waits_json(bir, maxw=1):
    """This neuronxcc walrus accepts at most one sync-wait per instruction;
    move excess waits onto preceding wait-only EventSemaphore ops."""
    uid = [0]
    for fn in bir["functions"]:
        for blk in fn["blocks"]:
            out = []
            for inst in blk["instructions"]:
                si = inst.get("sync_info")
                if si and si.get("on_wait") and len(si["on_wait"]) > maxw:
                    waits = si["on_wait"]
                    extra, keep = waits[:-maxw], waits[-maxw:]
                    for i in range(0, len(extra), maxw):
                        uid[0] += 1
                        out.append({
                            "debug": inst.get("debug", 0),
                            "engine": inst["engine"],
                            "ins": [],
                            "name": f"I-ws-{uid[0]}",
                            "opcode": "EventSemaphore",
                            "outs": [],
                            "sync_info": {"on_update": [],
                                          "on_wait": extra[i:i + maxw]},
                        })
                    si["on_wait"] = keep
                out.append(inst)
            blk["instructions"] = out
    return bir


# ------------------------------------------------------------ device build
def build_nc(plan, repeat=1):
    import concourse.bass as bass
    import concourse.mybir as mybir
    from concourse.tile import TileContext

    class PatchedBass(bass.Bass):
        def to_json_bytes(self):
            data = json.loads(super().to_json_bytes())
            return json.dumps(split_waits_json(data, 1)).encode()

    nc = PatchedBass()
    x_in = nc.dram_tensor("bev", [PLANES, H_B, W_B], mybir.dt.float32,
                          kind="ExternalInput")
    # unused input whose shape varies with `repeat`: defeats the NEFF cache's
    # shape-only HLO hash so timing variants compile separately
    nc.dram_tensor("nonce", [1, max(1, repeat)], mybir.dt.float32,
                   kind="ExternalInput")
    out = nc.dram_tensor("out", [PLANES, H_C, W_C], mybir.dt.float32,
                         kind="ExternalOutput")
    s_const = nc.inline_tensor(plan["S"], name="s_const")
    ident = nc.inline_tensor(np.tile(np.eye(64, dtype=np.float32), (2, 1)), name="ident")

    hl = plan["hl"]
    nslots = len(plan["used"])
    QW = W_C // NQ

    with TileContext(nc) as tc:
        with tc.tile_pool(name="sb", bufs=1) as pool, \
             tc.tile_pool(name="pst", bufs=4, space="PSUM") as pst, \
             tc.tile_pool(name="psc", bufs=1, space="PSUM") as psc:
            x_sb = pool.tile([128, hl], mybir.dt.float32)
            xt_sb = pool.tile([128, nslots * 64], mybir.dt.float32)
            s_sb = pool.tile([CHUNK, plan["sum_m"]], mybir.dt.float32)
            id_sb = pool.tile([128, 64], mybir.dt.float32)
            zero_sb = pool.tile([128, 1], mybir.dt.float32)
            col_sb = pool.tile([PLANES, W_C], mybir.dt.float32)
            col_int = pool.tile([128, W_C], mybir.dt.float32)

            nc.gpsimd.dma_start(s_sb[:], s_const[:])
            nc.gpsimd.dma_start(id_sb[:], ident[:])
            nc.gpsimd.memset(zero_sb[:], 0.0)

            # pad/garbage regions of the ring buffer must read as zeros
            for _rep in range(repeat):
              for hh in (0, 1):
                  pad0 = plan["half_len"][hh]
                  # find last rect end of this half
                  last = 0
                  for (h2, off, ya, h, xa, w) in plan["rect_dmas"]:
                      if h2 == hh:
                          last = max(last, off + h * w)
                  if hl > last:
                      nc.gpsimd.memset(x_sb[64 * hh:64 * hh + 64, last:hl], 0.0)

              # ring cover loads (issue split across HWDGE and SWDGE)
              for ri, (hh, off, ya, h, xa, w) in enumerate(plan["rect_dmas"]):
                  eng = nc.sync if ri % 3 != 2 else nc.gpsimd
                  eng.dma_start(
                      x_sb[64 * hh:64 * hh + 64, off:off + h * w],
                      x_in[:, ya:ya + h, xa:xa + w])

              # per-chunk PE transpose + copyback
              for c in plan["used"]:
                  hh, lo = plan["chunk_loc"](c)
                  sl = plan["slot"][c]
                  xt_ps = pst.tile([128, 64], mybir.dt.float32, name=f"xtps{c}",
                                   tag="xtps")
                  nc.tensor.transpose(
                      xt_ps[:], x_sb[64 * hh:64 * hh + 64, lo:lo + CHUNK],
                      id_sb[64 * hh:64 * hh + 64, :])
                  nc.vector.tensor_copy(xt_sb[:, 64 * sl:64 * sl + 64], xt_ps[:])

              # col accumulation: one PSUM tile (1 bank if NQ=4) per output
              # quarter so each quarter's copyback+DMA unblocks independently
              col_tiles = []
              mm_by_q = [[] for _ in range(NQ)]
              for i, (c, pst_, plen, so) in enumerate(plan["plan_mms"]):
                  mm_by_q[pst_ // QW].append((c, pst_, plen, so))
              for q in range(NQ):
                  col_q = psc.tile([PLANES, QW], mybir.dt.float32,
                                   name=f"colq{q}", tag=f"colq{q}")
                  col_tiles.append(col_q)
                  nc.tensor.matmul(
                      col_q[:],
                      zero_sb[:1, :1].to_broadcast((1, PLANES)),
                      zero_sb[:1, :1].to_broadcast((1, QW)),
                      start=True, stop=False, skip_group_check=True)
              for q in range(NQ):
                  col_q = col_tiles[q]
                  nmq = len(mm_by_q[q])
                  for i, (c, pst_, plen, so) in enumerate(mm_by_q[q]):
                      sl = plan["slot"][c]
                      nc.tensor.matmul(
                          col_q[:, pst_ - QW * q:pst_ - QW * q + plen],
                          xt_sb[:, 64 * sl:64 * sl + 64],
                          s_sb[:, so:so + plen],
                          start=False, stop=(i == nmq - 1),
                          skip_group_check=True)
              # copyback to partitions 0-63, duplicate into 64-127 so the
              # broadcast output DMA reads all 128 SBUF partitions (full
              # port bandwidth); dst partition dim = (z-half, plane)
              for q in range(NQ):
                  qs = slice(QW * q, QW * (q + 1))
                  cp = nc.vector.tensor_copy if q % 2 == 0 else nc.scalar.copy
                  cp(col_sb[:, qs], col_tiles[q][:])
                  # interleave col into (plane, zh) partition pairs so the
                  # output DMA reads all 128 SBUF partitions
                  nc.scalar.dma_start(col_int[0::2, qs], col_sb[:, qs])
                  nc.scalar.dma_start(col_int[1::2, qs], col_sb[:, qs])
                  srcb = col_int[:, None, qs].to_broadcast(
                      (128, H_C // 2, QW))
                  # dst: partition = plane*2+zh (uniform stride), then z, w
                  dst = bass.AP(out, QW * q, [
                      [H_C // 2 * W_C, 128],
                      [W_C, H_C // 2],
                      [1, QW],
                  ])
                  nc.sync.dma_start(dst, srcb)
    return nc


# ------------------------------------------------------------------ runner
def _get_state():
    if "state" in _CACHE:
        return _CACHE["state"]
    import jax
    import concourse.mybir as mybir
    from concourse import bass2jax
    from jax.sharding import Mesh, PartitionSpec
    from jax.experimental.shard_map import shard_map

    plan = build_plan()
    nc = build_nc(plan)
    bass2jax.install_neuronx_cc_hook()

    partition_name = (nc.partition_id_tensor.name
                      if nc.partition_id_tensor else None)
    in_names, out_names, out_avals, zero_outs = [], [], [], []
    for alloc in nc.m.functions[0].allocations:
        if not isinstance(alloc, mybir.MemoryLocationSet):
            continue
        name = alloc.memorylocations[0].name
        if alloc.kind == "ExternalInput":
            if name != partition_name:
                in_names.append(name)
        elif alloc.kind == "ExternalOutput":
            shape = tuple(alloc.tensor_shape)
            dtype = mybir.dt.np(alloc.dtype)
            out_names.append(name)
            out_avals.append(jax.core.ShapedArray(shape, dtype))
            zero_outs.append(np.zeros(shape, dtype))
    n_params = len(in_names)
    n_outs = len(out_names)
    all_names = in_names + out_names
    if partition_name is not None:
        all_names = all_names + [partition_name]
    donate = tuple(range(n_params, n_params + n_outs))

    def _body(*args):
        operands = list(args)
        if partition_name is not None:
            operands.append(bass2jax.partition_id_tensor())
        outs = bass2jax._bass_exec_p.bind(
            *operands,
            out_avals=tuple(out_avals),
            in_names=tuple(all_names),
            out_names=tuple(out_names),
            lowering_input_output_aliases=(),
            sim_require_finite=True,
            sim_require_nnan=True,
            nc=nc,
        )
        return tuple(outs)

    devices = jax.devices()[:NCORES]
    mesh = Mesh(np.asarray(devices), ("core",))
    specs = (PartitionSpec("core"),) * (n_params + n_outs)
    out_specs = (PartitionSpec("core"),) * n_outs
    fn = jax.jit(
        shard_map(_body, mesh=mesh, in_specs=specs, out_specs=out_specs,
                  check_rep=False),
        donate_argnums=donate, keep_unused=True)

    nonce = np.zeros((NCORES, 1), np.float32)
    state = dict(fn=fn, zero_outs=zero_outs, prev=None, nc=nc, plan=plan,
                 nonce=nonce)
    _CACHE["state"] = state
    return state


def kernel(bev_feat):
    bev = np.ascontiguousarray(np.asarray(bev_feat, dtype=np.float32))
    st = _get_state()
    global_in = bev.reshape(B * C, H_B, W_B)  # cores split axis 0: 64 each
    if st["prev"] is not None:
        zouts = st["prev"]          # donate previous device outputs
    else:
        zouts = [np.zeros((NCORES * z.shape[0], *z.shape[1:]), z.dtype)
                 for z in st["zero_outs"]]
    outs = st["fn"](global_in, st["nonce"], *zouts)
    result = np.asarray(outs[0])    # [512, 64, 2048]
    st["prev"] = list(outs)
    return result.reshape(B, C, H_C, W_C)



# revision 5
# speedup vs baseline: 46.1776x; 46.1776x over previous
"""BEV->Cylinder bilinear ring-sampling kernel for 8 Trainium2 NeuronCores.

Design (NTFF-profiled 140us vs 250us for the prior baseline):
  * f16 output path: col is cast to f16 at PSUM copyback; the 33.5MB/core
    output DMA becomes 16.8MB as 2KB full-contiguity descriptors. The host
    upcasts to f32 (rel-err ~7e-4 << 2e-2 tolerance).
  * Adaptive ring cover: per-row corner clusters chained along the circle,
    cut into rects by a DP minimizing rect-dispatch + covered-pixel cost;
    halves balanced by area (row-splitting the straddling rect).
  * Per-chunk [64->128] PE transpose feeding f16 matmuls (one per chunk and
    contiguous p-span, clipped at PSUM bank boundaries); measured PE cost is
    ~200ns fixed per instruction so fewer+wider matmuls win.
  * Pipelined emission: each chunk's matmuls follow its transpose in the PE
    stream, ring half 0 first; each output half's copyback/interleave/DMA is
    emitted right after its last matmul so output overlaps remaining loads.
  * Rect loads greedily cost-balanced across Pool(SWDGE ~0.34ns/desc gen) and
    SP/Act (HWDGE ~2ns/desc gen); output halves on the two HWDGE queues.
"""
import json
import math

import numpy as np

B, C, H_B, W_B = 4, 128, 512, 512
H_C, W_C = 64, 2048
MAX_RANGE = 50.0
XMIN, XMAX, YMIN, YMAX = -50.0, 50.0, -50.0, 50.0
NCORES = 8
PLANES = B * C // NCORES  # 64 planes per core

CHUNK = 128       # ring pixels per PE-transpose chunk
GAP = 32          # split x-clusters when gap exceeds this
RECT_NS = 1800.0  # modeled cost of one extra rect DMA (dispatch, serial)
PX_NS = 6.5       # modeled cost per covered ring pixel (DMA+PE+DVE)
P_GAP = 384       # split p-interval when gap exceeds this
NQ = 2            # output column halves (2KB f16 descriptors)
QW = W_C // NQ

_CACHE = {}


# ----------------------------------------------------------------- geometry
def _sample_xy():
    try:
        import jax
        import jax.numpy as jnp
        cpu = jax.devices("cpu")[0]
        with jax.default_device(cpu):
            phi = jnp.linspace(-math.pi, math.pi, W_C)
            x_g = MAX_RANGE * jnp.cos(phi)
            y_g = MAX_RANGE * jnp.sin(phi)
            x = (x_g - XMIN) / (XMAX - XMIN) * (W_B - 1)
            y = (YMAX - y_g) / (YMAX - YMIN) * (H_B - 1)
            return np.asarray(x, np.float32), np.asarray(y, np.float32)
    except Exception:
        phi = np.linspace(-math.pi, math.pi, W_C, dtype=np.float32)
        x_g = (MAX_RANGE * np.cos(phi)).astype(np.float32)
        y_g = (MAX_RANGE * np.sin(phi)).astype(np.float32)
        x = ((x_g - XMIN) / (XMAX - XMIN) * (W_B - 1)).astype(np.float32)
        y = ((YMAX - y_g) / (YMAX - YMIN) * (H_B - 1)).astype(np.float32)
        return x, y


def _corners():
    x, y = _sample_xy()
    x0 = np.floor(x).astype(np.int64)
    y0 = np.floor(y).astype(np.int64)
    wx1 = (x - x0.astype(np.float32)).astype(np.float32)
    wx0 = (np.float32(1.0) - wx1).astype(np.float32)
    wy1 = (y - y0.astype(np.float32)).astype(np.float32)
    wy0 = (np.float32(1.0) - wy1).astype(np.float32)
    out = []
    for xi, wx in ((x0, wx0), (x0 + 1, wx1)):
        for yi, wy in ((y0, wy0), (y0 + 1, wy1)):
            w = (wx * wy).astype(np.float32)
            valid = (xi >= 0) & (xi < W_B) & (yi >= 0) & (yi < H_B)
            for p in range(W_C):
                if valid[p]:
                    out.append((p, int(yi[p]), int(xi[p]), float(w[p])))
    return out


def _adaptive_cover(corner_list):
    """Cover the ring's corner pixels with rects. Per-row x-clusters are
    linked into vertical chains following the circle; each chain is cut into
    rects by a DP minimizing RECT_NS * n_rects + PX_NS * covered_px.
    Returns list of rects (ya, h, xa, w)."""
    rows = {}
    for p, yy, xx, w in corner_list:
        rows.setdefault(yy, set()).add(xx)
    clusters = {}
    for yy, xs in rows.items():
        xs = sorted(xs)
        cl = []
        st = prev = xs[0]
        for x in xs[1:] + [None]:
            if x is None or x - prev > GAP:
                cl.append((st, prev + 1))
                if x is not None:
                    st = x
            if x is not None:
                prev = x
        clusters[yy] = cl

    # link clusters into chains of consecutive rows by x-overlap
    chains = []        # list of list[(yy, xa, xb)]
    open_ch = []       # (chain_idx, xa, xb, last_y)
    for yy in sorted(clusters):
        live = [c for c in open_ch if c[3] == yy - 1]
        used = [False] * len(live)
        new_open = []
        for (xa, xb) in clusters[yy]:
            best, best_ov = None, -GAP - 1
            for i, (ci, lxa, lxb, _) in enumerate(live):
                if used[i]:
                    continue
                ov = min(xb, lxb) - max(xa, lxa)
                if ov > best_ov:
                    best, best_ov = i, ov
            if best is not None and best_ov >= -GAP:
                ci = live[best][0]
                used[best] = True
            else:
                ci = len(chains)
                chains.append([])
            chains[ci].append((yy, xa, xb))
            new_open.append((ci, xa, xb, yy))
        open_ch = new_open

    # DP cut per chain
    rects = []
    for ch in chains:
        n = len(ch)
        INF = float("inf")
        dp = [0.0] + [INF] * n
        back = [0] * (n + 1)
        for j in range(1, n + 1):
            mina, maxb = ch[j - 1][1], ch[j - 1][2]
            for i in range(j - 1, -1, -1):
                mina = min(mina, ch[i][1])
                maxb = max(maxb, ch[i][2])
                cost = dp[i] + RECT_NS + PX_NS * (j - i) * (maxb - mina)
                if cost < dp[j]:
                    dp[j], back[j] = cost, i
        j = n
        while j > 0:
            i = back[j]
            ya, yb = ch[i][0], ch[j - 1][0] + 1
            xa = min(c[1] for c in ch[i:j])
            xb = max(c[2] for c in ch[i:j])
            rects.append((ya, yb - ya, xa, xb - xa))
            j = i
    return rects


def _balance_split(rects):
    """Split the p-ordered rect list into two near-equal-area halves,
    splitting the straddling rect by rows if needed."""
    areas = [h * w for (ya, h, xa, w) in rects]
    total = sum(areas)
    cum = 0
    for i, a in enumerate(areas):
        if cum + a >= total / 2:
            need = int(round((total / 2 - cum) / rects[i][3]))
            ya, h, xa, w = rects[i]
            if 0 < need < h:
                first = rects[:i] + [(ya, need, xa, w)]
                second = [(ya + need, h - need, xa, w)] + rects[i + 1:]
            elif need <= 0:
                first, second = rects[:i], rects[i:]
            else:
                first, second = rects[:i + 1], rects[i + 1:]
            return [first, second]
        cum += a
    return [rects, []]


def build_plan():
    corner_list = _corners()
    rects = _adaptive_cover(corner_list)

    # order rects along the sampling circle (p order)
    def _rect_p(rc):
        ya, h, xa, w = rc
        yc, xc = ya + h / 2.0, xa + w / 2.0
        phi = math.atan2(255.5 - yc, xc - 255.5)
        return (phi + math.pi) / (2 * math.pi)
    rects.sort(key=_rect_p)

    # split rect list into two near-equal halves (partitions 0-63 / 64-127),
    # each padded to a CHUNK multiple
    halves = _balance_split(rects)
    rect_dmas = []   # (half, local_off, ya, h, xa, w)
    half_len = []    # padded length of each half
    last_data = []   # unpadded end of each half
    pix2ring = {}    # (y, x) -> (half, local ring position)
    for hh, rl in enumerate(halves):
        off = 0
        for (ya, h, xa, w) in rl:
            rect_dmas.append((hh, off, ya, h, xa, w))
            for r in range(h):
                for cc in range(w):
                    key = (ya + r, xa + cc)
                    if key not in pix2ring:
                        pix2ring[key] = (hh, off + r * w + cc)
            off += h * w
        last_data.append(off)
        off += (-off) % CHUNK
        half_len.append(off)
    hl = max(half_len)
    nch = [half_len[0] // CHUNK, half_len[1] // CHUNK]
    npairs = max(nch)

    # S weights per (half, local chunk) / p-span
    chunk_hits = {}
    for p, yy, xx, w in corner_list:
        hh, rp = pix2ring[(yy, xx)]
        d = chunk_hits.setdefault((hh, rp // CHUNK), {}).setdefault(p, {})
        r = rp % CHUNK
        d[r] = d.get(r, 0.0) + w

    plan_mms = []   # (pair_i, half, p_start, plen, s_off)
    s_cols = []
    s_off = 0
    for (hh, ci) in sorted(chunk_hits, key=lambda k: (k[0], k[1])):
        hits = chunk_hits[(hh, ci)]
        ps = sorted(hits)
        st = prev = ps[0]
        ivs = []
        for p in ps[1:] + [None]:
            if p is None or p - prev > P_GAP:
                ivs.append((st, prev - st + 1))
                if p is not None:
                    st = p
            if p is not None:
                prev = p
        split_ivs = []
        BANK = 512  # PSUM bank: matmul moving dim may not cross it
        for (pst, plen) in ivs:
            a = pst
            while a < pst + plen:
                b = min(pst + plen, (a // BANK + 1) * BANK)
                split_ivs.append((a, b - a))
                a = b
        for (pst, plen) in split_ivs:
            plan_mms.append((ci, hh, pst, plen, s_off))
            for p in range(pst, pst + plen):
                s_cols.append((hh, ci, p, hits.get(p, {})))
            s_off += plen
    S = np.zeros((CHUNK, s_off), dtype=np.float16)
    for j, (hh, ci, p, rws) in enumerate(s_cols):
        for r, w in rws.items():
            S[r, j] += np.float16(w)

    return dict(rect_dmas=rect_dmas, half_len=half_len, last_data=last_data,
                hl=hl, nch=nch, npairs=npairs, S=S,
                plan_mms=plan_mms, sum_m=s_off)


# ------------------------------------------------------- walrus wait-split
def split_waits_json(bir, maxw=1):
    uid = [0]
    for fn in bir["functions"]:
        for blk in fn["blocks"]:
            out = []
            for inst in blk["instructions"]:
                si = inst.get("sync_info")
                if si and si.get("on_wait") and len(si["on_wait"]) > maxw:
                    waits = si["on_wait"]
                    extra, keep = waits[:-maxw], waits[-maxw:]
                    for i in range(0, len(extra), maxw):
                        uid[0] += 1
                        out.append({
                            "debug": inst.get("debug", 0),
                            "engine": inst["engine"],
                            "ins": [],
                            "name": f"I-ws-{uid[0]}",
                            "opcode": "EventSemaphore",
                            "outs": [],
                            "sync_info": {"on_update": [],
                                          "on_wait": extra[i:i + maxw]},
                        })
                    si["on_wait"] = keep
                out.append(inst)
            blk["instructions"] = out
    return bir


# ------------------------------------------------------------ device build
def build_nc(plan, repeat=1):
    import concourse.bass as bass
    import concourse.mybir as mybir
    from concourse.tile import TileContext

    class PatchedBass(bass.Bass):
        def to_json_bytes(self):
            data = json.loads(super().to_json_bytes())
            return json.dumps(split_waits_json(data, 1)).encode()

    nc = PatchedBass()
    x_in = nc.dram_tensor("bev", [PLANES, H_B, W_B], mybir.dt.float32,
                          kind="ExternalInput")
    nc.dram_tensor("nonce", [1, max(1, repeat)], mybir.dt.float32,
                   kind="ExternalInput")
    out = nc.dram_tensor("out", [PLANES, H_C, W_C], mybir.dt.float16,
                         kind="ExternalOutput")
    s_const = nc.inline_tensor(plan["S"], name="s_const")
    ident = nc.inline_tensor(np.tile(np.eye(64, dtype=np.float32), (2, 1)),
                             name="ident")

    hl = plan["hl"]
    npairs = plan["npairs"]

    with TileContext(nc) as tc:
        with tc.tile_pool(name="sb", bufs=1) as pool, \
             tc.tile_pool(name="pst", bufs=4, space="PSUM") as pst, \
             tc.tile_pool(name="psc", bufs=1, space="PSUM") as psc:
            x_sb = pool.tile([128, hl], mybir.dt.float32)
            xt_sb = pool.tile([128, npairs * 128], mybir.dt.float16)
            s_sb = pool.tile([CHUNK, plan["sum_m"]], mybir.dt.float16)
            id_sb = pool.tile([128, 64], mybir.dt.float32)
            zero_sb = pool.tile([128, 1], mybir.dt.float16)
            col16 = pool.tile([PLANES, W_C], mybir.dt.float16)
            col_int = pool.tile([128, W_C], mybir.dt.float16)

            nc.gpsimd.dma_start(s_sb[:], s_const[:])
            nc.gpsimd.dma_start(id_sb[:], ident[:])
            nc.gpsimd.memset(zero_sb[:], 0.0)

            for _rep in range(repeat):
              # pad/garbage regions of the ring buffer must read as zeros
              for hh in (0, 1):
                  last = plan["last_data"][hh]
                  if hl > last:
                      nc.gpsimd.memset(x_sb[64 * hh:64 * hh + 64, last:hl], 0.0)

              # ring cover loads: greedy min-load assignment across the three
              # DMA-capable sequencers using measured gen costs. Pool (SWDGE,
              # 0.34ns/desc) absorbs descriptor-heavy rects; SP/Act (HWDGE,
              # ~2ns/desc) are pre-loaded with their later output-half gen.
              engs = [nc.gpsimd, nc.sync, nc.scalar]
              seq_load = [3000.0, 9000.0, 15000.0]
              for (hh, off, ya, h, xa, w) in plan["rect_dmas"]:
                  descs = 64 * h
                  costs = [994 + 0.4 * descs + 0.5 * h * w,
                           650 + 2.0 * descs,
                           650 + 2.0 * descs]
                  j = min(range(3), key=lambda k: seq_load[k] + costs[k])
                  seq_load[j] += costs[j]
                  engs[j].dma_start(
                      x_sb[64 * hh:64 * hh + 64, off:off + h * w],
                      x_in[:, ya:ya + h, xa:xa + w])

              # PSUM col tiles + zero-init (one accumulation group per half)
              col_tiles = []
              for q in range(NQ):
                  col_q = psc.tile([PLANES, QW], mybir.dt.float32,
                                   name=f"colq{q}", tag=f"colq{q}")
                  col_tiles.append(col_q)
                  for zb in range(0, QW, 512):
                      nc.tensor.matmul(
                          col_q[:, zb:zb + 512],
                          zero_sb[:1, :1].to_broadcast((1, PLANES)),
                          zero_sb[:1, :1].to_broadcast((1, 512)),
                          start=True, stop=False, skip_group_check=True)

              # per-chunk transpose -> f16 cast -> that chunk's matmuls, ring
              # half 0 first (its rects load first). Engine streams execute
              # in program order, so each output half's copyback/interleave/
              # DMA is emitted IMMEDIATELY after its last matmul — output
              # half 0 launches while ring half 1 is still loading.
              mm_by_chunk = {}
              last_mm_of_q = {}
              for k, mm in enumerate(plan["plan_mms"]):
                  ci, hh, pst_, plen, so = mm
                  mm_by_chunk.setdefault((hh, ci), []).append((k, mm))
                  last_mm_of_q[pst_ // QW] = k

              def emit_out_half(q):
                  qs = slice(QW * q, QW * (q + 1))
                  nc.vector.tensor_copy(col16[:, qs], col_tiles[q][:])
                  nc.gpsimd.dma_start(col_int[0::2, qs], col16[:, qs])
                  nc.gpsimd.dma_start(col_int[1::2, qs], col16[:, qs])
                  srcb = col_int[:, None, qs].to_broadcast(
                      (128, H_C // 2, QW))
                  dst = bass.AP(out, QW * q, [
                      [H_C // 2 * W_C, 128],
                      [W_C, H_C // 2],
                      [1, QW],
                  ])
                  eng = nc.sync if q % 2 == 0 else nc.scalar
                  eng.dma_start(dst, srcb)

              for hh in (0, 1):
                  for ci in range(plan["nch"][hh]):
                      lo = ci * CHUNK
                      slot = ci * 2 + hh
                      xt_ps = pst.tile([128, 64], mybir.dt.float32,
                                       name=f"xtps{hh}_{ci}", tag="xtps")
                      nc.tensor.transpose(
                          xt_ps[:], x_sb[64 * hh:64 * hh + 64, lo:lo + CHUNK],
                          id_sb[64 * hh:64 * hh + 64, :])
                      nc.vector.tensor_copy(
                          xt_sb[:, 64 * slot:64 * slot + 64], xt_ps[:])
                      for (k, (ci_, hh_, pst_, plen, so)) in mm_by_chunk.get(
                              (hh, ci), []):
                          q = pst_ // QW
                          nc.tensor.matmul(
                              col_tiles[q][:, pst_ - QW * q:
                                           pst_ - QW * q + plen],
                              xt_sb[:, 64 * slot:64 * slot + 64],
                              s_sb[:, so:so + plen],
                              start=False, stop=(k == last_mm_of_q[q]),
                              skip_group_check=True)
                          if k == last_mm_of_q[q]:
                              emit_out_half(q)
    return nc


# ------------------------------------------------------------------ runner
def _get_state():
    if "state" in _CACHE:
        return _CACHE["state"]
    import jax
    import concourse.mybir as mybir
    from concourse import bass2jax
    from jax.sharding import Mesh, PartitionSpec
    from jax.experimental.shard_map import shard_map

    plan = build_plan()
    nc = build_nc(plan)
    bass2jax.install_neuronx_cc_hook()

    partition_name = (nc.partition_id_tensor.name
                      if nc.partition_id_tensor else None)
    in_names, out_names, out_avals, zero_outs = [], [], [], []
    for alloc in nc.m.functions[0].allocations:
        if not isinstance(alloc, mybir.MemoryLocationSet):
            continue
        name = alloc.memorylocations[0].name
        if alloc.kind == "ExternalInput":
            if name != partition_name:
                in_names.append(name)
        elif alloc.kind == "ExternalOutput":
            shape = tuple(alloc.tensor_shape)
            dtype = mybir.dt.np(alloc.dtype)
            out_names.append(name)
            out_avals.append(jax.core.ShapedArray(shape, dtype))
            zero_outs.append(np.zeros(shape, dtype))
    n_params = len(in_names)
    n_outs = len(out_names)
    all_names = in_names + out_names
    if partition_name is not None:
        all_names = all_names + [partition_name]
    donate = tuple(range(n_params, n_params + n_outs))

    def _body(*args):
        operands = list(args)
        if partition_name is not None:
            operands.append(bass2jax.partition_id_tensor())
        outs = bass2jax._bass_exec_p.bind(
            *operands,
            out_avals=tuple(out_avals),
            in_names=tuple(all_names),
            out_names=tuple(out_names),
            lowering_input_output_aliases=(),
            sim_require_finite=True,
            sim_require_nnan=True,
            nc=nc,
        )
        return tuple(outs)

    devices = jax.devices()[:NCORES]
    mesh = Mesh(np.asarray(devices), ("core",))
    specs = (PartitionSpec("core"),) * (n_params + n_outs)
    out_specs = (PartitionSpec("core"),) * n_outs
    fn = jax.jit(
        shard_map(_body, mesh=mesh, in_specs=specs, out_specs=out_specs,
                  check_rep=False),
        donate_argnums=donate, keep_unused=True)

    nonce = np.zeros((NCORES, 1), np.float32)
    state = dict(fn=fn, zero_outs=zero_outs, prev=None, nc=nc, plan=plan,
                 nonce=nonce)
    _CACHE["state"] = state
    return state


def kernel(bev_feat):
    bev = np.ascontiguousarray(np.asarray(bev_feat, dtype=np.float32))
    st = _get_state()
    global_in = bev.reshape(B * C, H_B, W_B)  # cores split axis 0: 64 each
    if st["prev"] is not None:
        zouts = st["prev"]          # donate previous device outputs
    else:
        zouts = [np.zeros((NCORES * z.shape[0], *z.shape[1:]), z.dtype)
                 for z in st["zero_outs"]]
    outs = st["fn"](global_in, st["nonce"], *zouts)
    result = np.asarray(outs[0])    # [512, 64, 2048] f16
    st["prev"] = list(outs)
    return result.astype(np.float32).reshape(B, C, H_C, W_C)
